# revision 9
# baseline (speedup 1.0000x reference)
"""FlowNetC correlation (max_disp=20, stride2=2) Trainium2 Bass kernel.

Full inputs: input1, input2 [8, 256, 64, 128] f32.
Output: [8, 441, 64, 128] f32 where
  out[b, dj*21+di, y, x] = mean_c in1[b,c,y,x] * in2[b,c, y+2dj-20, x+2di-20]
(zero-filled where the shifted index is out of bounds).

Sharding: pure data parallelism, one batch element per NeuronCore (8 cores).

Per-core algorithm: displacements are stride-2, so y/x parity is preserved ->
4 independent phase sub-problems, each a unit-stride +-10 correlation on a
[256, 32, 64] image. Row-correlations are 21-diagonal bands of 64x64 Gram
matrices over C=256, computed on TensorE in fp32r. Blocks are pair-centric:
for each pair of in1 rows (m = 2x64 on PSUM partitions) the rhs covers the
pair's whole +-10 in2 row window (n <= 22*64, chunked <= 512). Band-diagonal
extraction cannot be expressed on-chip (SBUF access patterns cannot encode
per-partition offsets), so each pair's Gram strip is dumped to DRAM and the
bands re-read with a skewed (diagonal) flat-DRAM access pattern - one DMA per
output row. TensorE transposes put channels on partitions; a VectorE scaled
copy interleaves the two x-parities and applies the 1/256 mean; each output
row stores with 512B-contiguous runs.
"""

import os

import numpy as np

import concourse.bass as bass
import concourse.mybir as mybir
from concourse import bacc
from concourse.bass_utils import run_bass_kernel_spmd
from concourse.masks import make_identity
from concourse.tile import TileContext

B, C, H, W = 8, 256, 64, 128
DS, DR = 21, 10  # displacements per axis, radius
HH, XW = H // 2, W // 2  # per-phase dims: 32 rows, 64 cols
NCH = DS * DS  # 441 output channels
CHP = 448  # padded channel dim (4 transpose chunks of 112)
GPAD = 16  # flat margin: skew reads reach 10 elems outside a row section
MAXW = 2 * DR + 2  # max in2-row window per pair = 22
MAXGF = MAXW * XW  # 1408: max Gram strip free width


def _chunks(n):
    """Split n rows into balanced chunks of <= 8 (n*64 <= 512 per matmul) and
    >= 4 (fp32r keeps full rate at moving dim >= 256)."""
    k = -(-n // 8)
    base, rem = divmod(n, k)
    return [base + (1 if i < rem else 0) for i in range(k)]


def build_nc():
    skips = set(os.environ.get("CORR_SKIP", "").split(","))
    nc = bacc.Bacc("TRN2", target_bir_lowering=False, debug=False, num_devices=1)
    in1 = nc.dram_tensor("in1", [C, H, W], mybir.dt.float32, kind="ExternalInput")
    in2 = nc.dram_tensor("in2", [C, H, W], mybir.dt.float32, kind="ExternalInput")
    out = nc.dram_tensor("out", [NCH, H, W], mybir.dt.float32, kind="ExternalOutput")
    out_t = out.ap().tensor

    FREE = 2 * HH * W  # 8192: free size of each py-packed input tile

    with TileContext(nc) as tc:
        with (
            tc.tile_pool(name="persist", bufs=1) as persist,
            tc.tile_pool(name="gstage", bufs=3) as gstage,
            tc.tile_pool(name="band", bufs=3) as bandp,
            tc.tile_pool(name="outp", bufs=4) as outp,
            tc.tile_pool(name="psum_g", bufs=3, space="PSUM") as psg,
            tc.tile_pool(name="psum_t", bufs=4, space="PSUM") as pst,
            tc.tile_pool(name="gdump", bufs=40, space="DRAM") as gdump,
        ):
            # ---- load inputs y-parity-packed: per py a tile [ci=128, co=2, yy=32, x=128]
            # (c = co*128 + ci, y = 2*yy + py). In this layout a matmul operand over
            # consecutive packed rows at one x-parity is a single stride-2
            # progression (row step 128 = 64*2).
            in_sb = {}
            for name, src in (("i1", in1), ("i2", in2)):
                for py in range(2):
                    t = persist.tile(
                        [128, 2, HH, W], mybir.dt.float32r, name=f"{name}p{py}"
                    )
                    for co in range(2):
                        nc.sync.dma_start(
                            t[:, co],
                            bass.AP(
                                tensor=src.ap().tensor,
                                offset=co * 128 * (H * W) + py * W,
                                ap=[[H * W, 128], [2 * W, HH], [1, W]],
                            ).bitcast(mybir.dt.float32r),
                        )
                    in_sb[(name, py)] = t

            ident = persist.tile([64, 64], mybir.dt.float32)
            make_identity(nc, ident[:])

            def operand(t, co, yy0, px, nrows):
                """fp32r matmul operand [128, nrows*64]: partitions ci; the
                (row, xx) pairs of nrows consecutive packed rows form a single
                stride-2 progression."""
                off = t.offset + co * (HH * W) + yy0 * W + px
                return bass.AP(
                    tensor=t.tensor, offset=off, ap=[[FREE, 128], [2, nrows * XW]]
                )

            for py in range(2):
                gtiles = {}
                winA = {}
                # 1) pair-centric Gram strips + one dump per pair
                for px in range(2):
                    for pi in range(HH // 2):
                        yy1 = 2 * pi
                        A = max(0, yy1 - DR)
                        Bw = min(HH - 1, yy1 + 1 + DR)
                        nW = Bw - A + 1
                        winA[pi] = A
                        gw = nW * XW
                        gt = gstage.tile([128, MAXGF], mybir.dt.float32, name="gt")
                        v0 = A
                        for cn in _chunks(nW):
                            pg = psg.tile([128, 512], mybir.dt.float32, name="pg")
                            for co in range(2):
                                if "mm" not in skips:
                                    nc.tensor.matmul(
                                        pg[:, : cn * XW],
                                        operand(in_sb[("i1", py)], co, yy1, px, 2),
                                        operand(in_sb[("i2", py)], co, v0, px, cn),
                                        start=(co == 0),
                                        stop=(co == 1),
                                    )
                            if "copyback" not in skips:
                                nc.scalar.copy(
                                    gt[:, (v0 - A) * XW : (v0 - A + cn) * XW],
                                    pg[:, : cn * XW],
                                )
                            v0 += cn
                        dt_ = gdump.tile(
                            [1, 128 * MAXGF + 2 * GPAD], mybir.dt.float32, name="dt"
                        )
                        if "dump" not in skips:
                            nc.sync.dma_start(
                                bass.AP(
                                    tensor=dt_.tensor,
                                    offset=dt_.offset + GPAD,
                                    ap=[[gw, 128], [1, gw]],
                                ),
                                gt[:, :gw],
                            )
                        gtiles[(px, pi)] = dt_

                # 2) per output row: one skew DMA per parity, transposes,
                #    interleave, store
                for yy in range(HH):
                    pi, yysel = yy // 2, yy % 2
                    A = winA[pi]
                    gw = (min(HH - 1, 2 * pi + 1 + DR) - A + 1) * XW
                    djlo = max(0, DR - yy)
                    djhi = min(DS - 1, DR + (HH - 1 - yy))
                    ndj = djhi - djlo + 1
                    sect0 = (yy + djlo - DR) - A
                    ot = outp.tile([112, 4, W], mybir.dt.float32, name="ot")
                    for px in range(2):
                        byy = bandp.tile([64, CHP], mybir.dt.float32, name="byy")
                        if "memset" not in skips:
                            nc.gpsimd.memset(byy[:], 0.0)
                        dt_ = gtiles[(px, pi)]
                        src = bass.AP(
                            tensor=dt_.tensor,
                            offset=dt_.offset + GPAD + yysel * 64 * gw + sect0 * XW - DR,
                            ap=[[gw + 1, 64], [XW, ndj], [1, DS]],
                        )
                        dst = bass.AP(
                            tensor=byy.tensor,
                            offset=byy.offset + djlo * DS,
                            ap=[[CHP, 64], [DS, ndj], [1, DS]],
                        )
                        if "skew" not in skips:
                            nc.sync.dma_start(dst, src)
                        # zero x-edge triangles: (xx, dj, di) valid iff
                        # 0 <= xx + di - 10 < 64
                        if "affine" not in skips:
                            nc.gpsimd.affine_select(
                                out=byy[:, :NCH],
                                in_=byy[:, :NCH],
                                compare_op=mybir.AluOpType.is_ge,
                                fill=0.0,
                                base=-DR,
                                pattern=[[0, DS], [1, DS]],
                                channel_multiplier=1,
                            )
                            nc.gpsimd.affine_select(
                                out=byy[:, :NCH],
                                in_=byy[:, :NCH],
                                compare_op=mybir.AluOpType.is_ge,
                                fill=0.0,
                                base=DR + (XW - 1),
                                pattern=[[0, DS], [-1, DS]],
                                channel_multiplier=-1,
                            )
                        for t in range(4):
                            nch = 112 if t < 3 else NCH - 336
                            pt = pst.tile([112, 64], mybir.dt.float32, name="pt")
                            if "transpose" not in skips:
                                nc.tensor.transpose(
                                    pt[:], byy[:, 112 * t : 112 * (t + 1)], ident[:]
                                )
                            dstv = bass.AP(
                                tensor=ot.tensor,
                                offset=ot.offset + t * W + px,
                                ap=[[4 * W, nch], [2, XW]],
                            )
                            if "inter" not in skips:
                                nc.vector.tensor_scalar_mul(dstv, pt[:nch, :], 1.0 / C)
                    if "store" not in skips:
                        # channels 0..336 in one 3D DMA; the partial last chunk
                        # (105 channels) separately to stay in bounds
                        nc.sync.dma_start(
                            bass.AP(
                                tensor=out_t,
                                offset=(2 * yy + py) * W,
                                ap=[[H * W, 112], [112 * H * W, 3], [1, W]],
                            ),
                            bass.AP(
                                tensor=ot.tensor,
                                offset=ot.offset,
                                ap=[[4 * W, 112], [W, 3], [1, W]],
                            ),
                        )
                        nc.sync.dma_start(
                            bass.AP(
                                tensor=out_t,
                                offset=336 * (H * W) + (2 * yy + py) * W,
                                ap=[[H * W, NCH - 336], [1, W]],
                            ),
                            bass.AP(
                                tensor=ot.tensor,
                                offset=ot.offset + 3 * W,
                                ap=[[4 * W, NCH - 336], [1, W]],
                            ),
                        )

    nc.compile()
    return nc


_NC_CACHE = None


def kernel(input1: np.ndarray, input2: np.ndarray) -> np.ndarray:
    global _NC_CACHE
    input1 = np.ascontiguousarray(input1, dtype=np.float32)
    input2 = np.ascontiguousarray(input2, dtype=np.float32)
    assert input1.shape == (B, C, H, W), input1.shape
    if _NC_CACHE is None:
        _NC_CACHE = build_nc()
    nc = _NC_CACHE
    in_maps = [dict(in1=input1[b], in2=input2[b]) for b in range(B)]
    res = run_bass_kernel_spmd(nc, in_maps, core_ids=list(range(B)))
    return np.stack([r["out"] for r in res.results], axis=0)


if __name__ == "__main__":
    rng = np.random.default_rng(0)
    i1 = rng.standard_normal((B, C, H, W), dtype=np.float32)
    i2 = rng.standard_normal((B, C, H, W), dtype=np.float32)
    o = kernel(i1, i2)
    print("out", o.shape, o.dtype, float(np.abs(o).max()))


# revision 15
# speedup vs baseline: 1.0715x; 1.0715x over previous
"""FlowNetC correlation (max_disp=20, stride2=2) Trainium2 Bass kernel.

Full inputs: input1, input2 [8, 256, 64, 128] f32.
Output: [8, 441, 64, 128] f32 where
  out[b, dj*21+di, y, x] = mean_c in1[b,c,y,x] * in2[b,c, y+2dj-20, x+2di-20]
(zero-filled where the shifted index is out of bounds).

Sharding: pure data parallelism, one batch element per NeuronCore (8 cores).

Per-core algorithm: displacements are stride-2, so y/x parity is preserved ->
4 independent phase sub-problems, each a unit-stride +-10 correlation on a
[256, 32, 64] image. Row-correlations are 21-diagonal bands of 64x64 Gram
matrices over C=256, computed on TensorE in fp32r. Blocks are pair-centric:
for each pair of in1 rows (m = 2x64 on PSUM partitions) the rhs covers the
pair's whole +-10 in2 row window (n <= 22*64, chunked <= 512). Band-diagonal
extraction cannot be expressed on-chip (SBUF access patterns cannot encode
per-partition offsets), so each pair's Gram strip is dumped to DRAM and the
bands re-read with a skewed (diagonal) flat-DRAM access pattern - one DMA per
output row. TensorE transposes put channels on partitions; a VectorE scaled
copy interleaves the two x-parities and applies the 1/256 mean; each output
row stores with 512B-contiguous runs.
"""

import os

import numpy as np

import concourse.bass as bass
import concourse.mybir as mybir
from concourse import bacc
from concourse.bass_utils import run_bass_kernel_spmd
from concourse.masks import make_identity
from concourse.tile import TileContext

B, C, H, W = 8, 256, 64, 128
DS, DR = 21, 10  # displacements per axis, radius
HH, XW = H // 2, W // 2  # per-phase dims: 32 rows, 64 cols
NCH = DS * DS  # 441 output channels
CHP = 448  # padded channel dim (4 transpose chunks of 112)
GPAD = 16  # flat margin: skew reads reach 10 elems outside a row section
MAXW = 2 * DR + 2  # max in2-row window per pair = 22
MAXGF = MAXW * XW  # 1408: max Gram strip free width


def _chunks(n):
    """Split n rows into balanced chunks of <= 8 (n*64 <= 512 per matmul) and
    >= 4 (fp32r keeps full rate at moving dim >= 256)."""
    k = -(-n // 8)
    base, rem = divmod(n, k)
    return [base + (1 if i < rem else 0) for i in range(k)]


def build_nc():
    skips = set(os.environ.get("CORR_SKIP", "").split(","))
    nc = bacc.Bacc("TRN2", target_bir_lowering=False, debug=False, num_devices=1)
    in1 = nc.dram_tensor("in1", [C, H, W], mybir.dt.float32, kind="ExternalInput")
    in2 = nc.dram_tensor("in2", [C, H, W], mybir.dt.float32, kind="ExternalInput")
    out = nc.dram_tensor("out", [NCH, H, W], mybir.dt.float32, kind="ExternalOutput")
    cmask = nc.dram_tensor("cmask", [4, 112, XW], mybir.dt.uint8, kind="ExternalInput")
    out_t = out.ap().tensor

    FREE = 2 * HH * W  # 8192: free size of each py-packed input tile

    with TileContext(nc) as tc:
        with (
            tc.tile_pool(name="persist", bufs=1) as persist,
            tc.tile_pool(name="gstage", bufs=3) as gstage,
            tc.tile_pool(name="band", bufs=3) as bandp,
            tc.tile_pool(name="outp", bufs=4) as outp,
            tc.tile_pool(name="psum_g", bufs=3, space="PSUM") as psg,
            tc.tile_pool(name="psum_t", bufs=4, space="PSUM") as pst,
            tc.tile_pool(name="gdump", bufs=40, space="DRAM") as gdump,
        ):
            # ---- load inputs y-parity-packed: per py a tile [ci=128, co=2, yy=32, x=128]
            # (c = co*128 + ci, y = 2*yy + py). In this layout a matmul operand over
            # consecutive packed rows at one x-parity is a single stride-2
            # progression (row step 128 = 64*2).
            in_sb = {}
            for name, src in (("i1", in1), ("i2", in2)):
                for py in range(2):
                    t = persist.tile(
                        [128, 2, HH, W], mybir.dt.float32r, name=f"{name}p{py}"
                    )
                    for co in range(2):
                        nc.sync.dma_start(
                            t[:, co],
                            bass.AP(
                                tensor=src.ap().tensor,
                                offset=co * 128 * (H * W) + py * W,
                                ap=[[H * W, 128], [2 * W, HH], [1, W]],
                            ).bitcast(mybir.dt.float32r),
                        )
                    in_sb[(name, py)] = t

            ident = persist.tile([64, 64], mybir.dt.float32)
            make_identity(nc, ident[:])
            # x-edge validity mask in channel-major form, scaled by 1/256:
            # cmask[t, p, xx] = (0 <= xx + ((112*t+p) % 21) - 10 < 64) / 256
            mask_sb = persist.tile([112, 4, XW], mybir.dt.uint8)
            nc.sync.dma_start(
                mask_sb[:],
                bass.AP(
                    tensor=cmask.ap().tensor,
                    offset=0,
                    ap=[[XW, 112], [112 * XW, 4], [1, XW]],
                ),
            )

            def operand(t, co, yy0, px, nrows):
                """fp32r matmul operand [128, nrows*64]: partitions ci; the
                (row, xx) pairs of nrows consecutive packed rows form a single
                stride-2 progression."""
                off = t.offset + co * (HH * W) + yy0 * W + px
                return bass.AP(
                    tensor=t.tensor, offset=off, ap=[[FREE, 128], [2, nrows * XW]]
                )

            for py in range(2):
                gtiles = {}
                winA = {}
                # 1) pair-centric Gram strips + one dump per pair
                for px in range(2):
                    for pi in range(HH // 2):
                        yy1 = 2 * pi
                        A = max(0, yy1 - DR)
                        Bw = min(HH - 1, yy1 + 1 + DR)
                        nW = Bw - A + 1
                        winA[pi] = A
                        gw = nW * XW
                        gt = gstage.tile([128, MAXGF], mybir.dt.float32, name="gt")
                        v0 = A
                        for cn in _chunks(nW):
                            pg = psg.tile([128, 512], mybir.dt.float32, name="pg")
                            for co in range(2):
                                if "mm" not in skips:
                                    nc.tensor.matmul(
                                        pg[:, : cn * XW],
                                        operand(in_sb[("i1", py)], co, yy1, px, 2),
                                        operand(in_sb[("i2", py)], co, v0, px, cn),
                                        start=(co == 0),
                                        stop=(co == 1),
                                    )
                            if "copyback" not in skips:
                                nc.scalar.mul(
                                    gt[:, (v0 - A) * XW : (v0 - A + cn) * XW],
                                    pg[:, : cn * XW],
                                    1.0 / C,
                                )
                            v0 += cn
                        dt_ = gdump.tile(
                            [1, 128 * MAXGF + 2 * GPAD], mybir.dt.float32, name="dt"
                        )
                        if "dump" not in skips:
                            nc.sync.dma_start(
                                bass.AP(
                                    tensor=dt_.tensor,
                                    offset=dt_.offset + GPAD,
                                    ap=[[gw, 128], [1, gw]],
                                ),
                                gt[:, :gw],
                            )
                        gtiles[(px, pi)] = dt_

                # 2) per output row: one skew DMA per parity, transposes,
                #    interleave, store
                for yy in range(HH):
                    pi, yysel = yy // 2, yy % 2
                    A = winA[pi]
                    gw = (min(HH - 1, 2 * pi + 1 + DR) - A + 1) * XW
                    djlo = max(0, DR - yy)
                    djhi = min(DS - 1, DR + (HH - 1 - yy))
                    ndj = djhi - djlo + 1
                    sect0 = (yy + djlo - DR) - A
                    ot = outp.tile([112, 4, W], mybir.dt.float32, name="ot")
                    if "memset" not in skips:
                        nc.gpsimd.memset(ot[:], 0.0)
                    for px in range(2):
                        byy = bandp.tile([64, CHP], mybir.dt.float32, name="byy")
                        if "memset" not in skips:
                            # only dj slots the skew DMA will not write + pad cols
                            if djlo > 0:
                                nc.gpsimd.memset(byy[:, : djlo * DS], 0.0)
                            nc.gpsimd.memset(byy[:, (djhi + 1) * DS :], 0.0)
                        dt_ = gtiles[(px, pi)]
                        src = bass.AP(
                            tensor=dt_.tensor,
                            offset=dt_.offset + GPAD + yysel * 64 * gw + sect0 * XW - DR,
                            ap=[[gw + 1, 64], [XW, ndj], [1, DS]],
                        )
                        dst = bass.AP(
                            tensor=byy.tensor,
                            offset=byy.offset + djlo * DS,
                            ap=[[CHP, 64], [DS, ndj], [1, DS]],
                        )
                        if "skew" not in skips:
                            nc.sync.dma_start(dst, src)
                        for t in range(4):
                            nch = 112 if t < 3 else NCH - 336
                            pt = pst.tile([112, 64], mybir.dt.float32, name="pt")
                            if "transpose" not in skips:
                                nc.tensor.transpose(
                                    pt[:], byy[:, 112 * t : 112 * (t + 1)], ident[:]
                                )
                            dstv = bass.AP(
                                tensor=ot.tensor,
                                offset=ot.offset + t * W + px,
                                ap=[[4 * W, nch], [2, XW]],
                            )
                            if "inter" not in skips:
                                nc.vector.copy_predicated(
                                    dstv, mask_sb[:nch, t, :], pt[:nch, :]
                                )
                    if "store" not in skips:
                        # channels 0..336 in one 3D DMA; the partial last chunk
                        # (105 channels) separately to stay in bounds
                        nc.sync.dma_start(
                            bass.AP(
                                tensor=out_t,
                                offset=(2 * yy + py) * W,
                                ap=[[H * W, 112], [112 * H * W, 3], [1, W]],
                            ),
                            bass.AP(
                                tensor=ot.tensor,
                                offset=ot.offset,
                                ap=[[4 * W, 112], [W, 3], [1, W]],
                            ),
                        )
                        nc.sync.dma_start(
                            bass.AP(
                                tensor=out_t,
                                offset=336 * (H * W) + (2 * yy + py) * W,
                                ap=[[H * W, NCH - 336], [1, W]],
                            ),
                            bass.AP(
                                tensor=ot.tensor,
                                offset=ot.offset + 3 * W,
                                ap=[[4 * W, NCH - 336], [1, W]],
                            ),
                        )

    nc.compile()
    return nc


_NC_CACHE = None


def kernel(input1: np.ndarray, input2: np.ndarray) -> np.ndarray:
    global _NC_CACHE
    input1 = np.ascontiguousarray(input1, dtype=np.float32)
    input2 = np.ascontiguousarray(input2, dtype=np.float32)
    assert input1.shape == (B, C, H, W), input1.shape
    if _NC_CACHE is None:
        _NC_CACHE = build_nc()
    nc = _NC_CACHE
    ch = np.arange(448) % DS
    xx = np.arange(XW)
    valid = (xx[None, :] + ch[:, None] - DR >= 0) & (xx[None, :] + ch[:, None] - DR < XW)
    cm = valid.astype(np.uint8).reshape(4, 112, XW)
    in_maps = [dict(in1=input1[b], in2=input2[b], cmask=cm) for b in range(B)]
    res = run_bass_kernel_spmd(nc, in_maps, core_ids=list(range(B)))
    return np.stack([r["out"] for r in res.results], axis=0)


if __name__ == "__main__":
    rng = np.random.default_rng(0)
    i1 = rng.standard_normal((B, C, H, W), dtype=np.float32)
    i2 = rng.standard_normal((B, C, H, W), dtype=np.float32)
    o = kernel(i1, i2)
    print("out", o.shape, o.dtype, float(np.abs(o).max()))
